# revision 1
# baseline (speedup 1.0000x reference)
"""Trainium2 Bass kernel: EquivariantLayerNorm (irreps 128x0e + 64x1o + 32x2e).

Math (per row x of 480 features; scalar channels = first 128):
    m    = mean(x[:128]);  x'[:128] = x[:128] - m;  x'[128:] = x[128:]
    ss   = sum(x'^2) = sum(x^2) - 128*m^2           (groups partition the row)
    r    = 1/sqrt(ss/224)
    y    = x' * r * w_full;  y[:128] += bias
The Invariant eps terms (eps=1e-6) contribute <1e-6 relative error and are
dropped (below fp32 rounding of the reference itself).

Sharding: pure data-parallel over the row dimension across 8 NeuronCores.
Each core gets 12500 rows, padded to 12544 = 98 blocks of 128 rows
(pad rows filled with 1.0 so all stats stay finite).

Per-core kernel layout: rows on partitions (128/block), features on the free
dim. Big tiles of G=14 blocks (1792 rows, 3.4MB) per DMA. Per block:
  ACT: accum(-x_A/128) -> -mean ; accum(square(x)) -> ss
  DVE (batched over G): -128*nm^2 ; (ss+t)/224 ; ACT sqrt ; DVE reciprocal
  DVE: x_A += nm (in place) ; x = (x * r) * w_bcast (fused STT, in place)
  DVE: x[:, :, :128] += bias (one op per big tile, stride-0 broadcast)
"""

import numpy as np

DIM = 480
NS = 128          # scalar (0e) channels
NF = 224          # irrep instances
BLK = 128         # rows per block (partition dim)
N_CORES = 8

N_TOTAL = 100000
ROWS_PER_CORE = N_TOTAL // N_CORES    # 12500
G = 14                                # blocks per big tile
NBLOCKS = 98                          # ceil(12500/128)
ROWS_PAD = NBLOCKS * BLK              # 12544
NTILES = NBLOCKS // G                 # 7


def _expand_w(affine_weight):
    return np.concatenate([
        affine_weight[0:128],
        np.repeat(affine_weight[128:192], 3),
        np.repeat(affine_weight[192:224], 5),
    ]).astype(np.float32)


def _split_excess_waits(nc, dummy_sem):
    """walrus' TRN2 codegen allows at most ONE sync-wait command per engine
    instruction (S3D3_*_STRUCT).  Tile's wait assignment can emit 2+ — move
    the excess onto standalone InstEventSemaphore no-ops (same engine, placed
    immediately before), which is the same mechanism Tile's own barriers use.
    Each carries a dead increment of ``dummy_sem`` (CoreSim requires updates).
    """
    from concourse import mybir

    n = 0
    for fn in nc.m.functions:
        for blk in fn.blocks:
            out = []
            changed = False
            for inst in blk.instructions:
                si = inst.sync_info
                if si is not None and si.on_wait and len(si.on_wait) > 1:
                    waits = list(si.on_wait)
                    for w in waits[:-1]:
                        n += 1
                        ev = mybir.InstEventSemaphore(
                            name=f"I-evsplit-{n}", ins=[], outs=[])
                        ev.engine = inst.engine
                        ev.sync_info = mybir.SyncInfo(
                            on_wait=[w],
                            on_update=[mybir.SyncUpdate(
                                sync_type="semaphore", id=dummy_sem.num,
                                ant_name=dummy_sem.name,
                                update_mode="sem-inc", update_value=1,
                                update_reg=None)])
                        out.append(ev)
                    inst.sync_info = mybir.SyncInfo(
                        on_wait=[waits[-1]], on_update=list(si.on_update or []))
                    changed = True
                out.append(inst)
            if changed:
                blk.instructions = out
    return n


def build_nc(rows_pad, g, data_bufs=5):
    import concourse.bacc as bacc
    import concourse.tile as tile
    from concourse import mybir
    # Loads issue from the SP HWDGE ring, stores from the ACT HWDGE ring —
    # separate FIFO rings, so store waits never block load issue. Pin one
    # completion-semaphore lane per ring (lane = ring order = FIFO-exact),
    # which also keeps consumers at one DMAHW wait per ring.
    from concourse import tile_sem_assignment as _tsa
    if not getattr(_tsa.TileClockTick, "_lane_by_engine", False):
        _orig_assign = _tsa.TileClockTick._assign_tick

        def _assign_tick_lane_by_engine(self, inst):
            if isinstance(inst, _tsa.DMAInst) and not isinstance(
                    inst, _tsa.bass_isa.UserSyncedRemoteDMADescs):
                if inst.engine == mybir.EngineType.SP:
                    self.next_hw_dma_idx = 0
                elif inst.engine == mybir.EngineType.Activation:
                    self.next_hw_dma_idx = 1
            return _orig_assign(self, inst)

        _tsa.TileClockTick._assign_tick = _assign_tick_lane_by_engine
        _tsa.TileClockTick._lane_by_engine = True

    f32 = mybir.dt.float32
    Alu = mybir.AluOpType
    Act = mybir.ActivationFunctionType

    nblocks = rows_pad // BLK
    assert rows_pad % BLK == 0
    if isinstance(g, int):
        assert nblocks % g == 0
        gs = [g] * (nblocks // g)
    else:
        gs = list(g)
        assert sum(gs) == nblocks
    ntiles = len(gs)

    nc = bacc.Bacc("TRN2", target_bir_lowering=False, debug=False)
    evsem = nc.alloc_semaphore("evsplit_dummy")
    x = nc.dram_tensor("x", [rows_pad, DIM], f32, kind="ExternalInput").ap()
    w = nc.dram_tensor("w", [1, DIM], f32, kind="ExternalInput").ap()
    b = nc.dram_tensor("b", [1, NS], f32, kind="ExternalInput").ap()
    y = nc.dram_tensor("y", [rows_pad, DIM], f32, kind="ExternalOutput").ap()

    with tile.TileContext(nc) as tc:
        with (
            tc.tile_pool(name="const", bufs=1) as cpool,
            tc.tile_pool(name="data", bufs=data_bufs) as dpool,
            tc.tile_pool(name="stats", bufs=ntiles) as spool,
            tc.tile_pool(name="scratch", bufs=1) as zpool,
        ):
            w_t = cpool.tile([BLK, DIM], f32, name="w_t")
            nc.sync.dma_start(out=w_t[:], in_=w.broadcast_to([BLK, DIM]))
            b_t = cpool.tile([BLK, NS], f32, name="b_t")
            nc.sync.dma_start(out=b_t[:], in_=b.broadcast_to([BLK, NS]))

            da = zpool.tile([BLK, NS], f32, name="da")     # dead store (ACT accum)
            df = zpool.tile([BLK, DIM], f32, name="df")    # dead store (ACT accum)

            r0 = 0
            for t, g in enumerate(gs):
                src = x[r0:r0 + g * BLK, :].rearrange("(g p) d -> p g d", p=BLK)
                xt = dpool.tile([BLK, g, DIM], f32, tag="xt", name=f"xt{t}")
                nc.sync.dma_start(out=xt[:], in_=src)

                # Tiny DVE read of the freshly loaded tile: absorbs the DMAHW
                # wait on DVE so the hot TensorScalar ops below stay at <=1
                # sync wait (the ISA TS struct rejects 2+).
                dv = spool.tile([BLK, 2], f32, tag="dv", name=f"dv{t}")
                nc.vector.tensor_copy(dv[:], xt[:, g - 1, DIM - 2:DIM])

                # Raw row-sums of the scalar block: a few blocks per tile on
                # DVE (tensor_reduce), the rest on ACT (Copy + accum), to
                # balance the two engines under the DMA-paced cadence.
                ndve = max(0, round(g * 2 / 7))
                nmr = spool.tile([BLK, g], f32, tag="nmr", name=f"nmr{t}")
                ss = spool.tile([BLK, g], f32, tag="ss", name=f"ss{t}")
                for j in range(g):
                    if j < ndve:
                        nc.vector.tensor_reduce(
                            out=nmr[:, j:j + 1], in_=xt[:, j, 0:NS],
                            axis=mybir.AxisListType.X, op=Alu.add)
                    else:
                        nc.scalar.activation(
                            out=da[:], in_=xt[:, j, 0:NS], func=Act.Copy,
                            scale=1.0, accum_out=nmr[:, j:j + 1])
                    # (x/sqrt(224))^2 accumulated -> sum(x^2)/224 directly
                    nc.scalar.activation(
                        out=df[:], in_=xt[:, j, :], func=Act.Square,
                        scale=1.0 / float(NF) ** 0.5,
                        accum_out=ss[:, j:j + 1])

                nm = spool.tile([BLK, g], f32, tag="nm", name=f"nm{t}")
                nc.vector.tensor_scalar(
                    out=nm[:], in0=nmr[:], scalar1=-1.0 / NS, scalar2=None,
                    op0=Alu.mult)
                tt = spool.tile([BLK, g], f32, tag="tt", name=f"tt{t}")
                nc.vector.scalar_tensor_tensor(
                    out=tt[:], in0=nm[:], scalar=-float(NS) / float(NF),
                    in1=nm[:], op0=Alu.mult, op1=Alu.mult)
                vv = spool.tile([BLK, g], f32, tag="vv", name=f"vv{t}")
                nc.vector.tensor_tensor(
                    out=vv[:], in0=ss[:], in1=tt[:], op=Alu.add)
                sq = spool.tile([BLK, g], f32, tag="sq", name=f"sq{t}")
                nc.scalar.activation(out=sq[:], in_=vv[:], func=Act.Sqrt)
                rr = spool.tile([BLK, g], f32, tag="rr", name=f"rr{t}")
                nc.vector.reciprocal(out=rr[:], in_=sq[:])

                for j in range(g):
                    blk = xt[:, j, :]
                    nc.vector.tensor_scalar(
                        out=xt[:, j, 0:NS], in0=xt[:, j, 0:NS],
                        scalar1=nm[:, j:j + 1], scalar2=None, op0=Alu.add)
                    nc.vector.scalar_tensor_tensor(
                        out=blk, in0=blk, scalar=rr[:, j:j + 1], in1=w_t[:],
                        op0=Alu.mult, op1=Alu.mult)

                bias_bc = b_t[:].unsqueeze(1).broadcast_to([BLK, g, NS])
                nc.vector.tensor_tensor(
                    out=xt[:, :, 0:NS], in0=xt[:, :, 0:NS], in1=bias_bc,
                    op=Alu.add)

                dst = y[r0:r0 + g * BLK, :].rearrange("(g p) d -> p g d", p=BLK)
                nc.scalar.dma_start(out=dst, in_=xt[:])
                r0 += g * BLK

    nc.compile()
    _split_excess_waits(nc, evsem)
    return nc


_NC_CACHE = {}

# Variable tile schedule: small tiles at the start (first stats begin after a
# 0.5MB load instead of 3.4MB) and at the end (short stats->apply->store tail
# after the final load); big tiles in the middle for DMA efficiency.
G_SCHEDULE = (17, 17, 16, 16, 16, 16)   # 6 near-uniform tiles; sums to 98


def _get_nc(rows_pad, g):
    key = (rows_pad, tuple(g) if not isinstance(g, int) else g)
    if key not in _NC_CACHE:
        _NC_CACHE[key] = build_nc(rows_pad, g)
    return _NC_CACHE[key]


def kernel(node_input, affine_weight, affine_bias):
    from concourse.bass_utils import run_bass_kernel_spmd

    node_input = np.ascontiguousarray(node_input, dtype=np.float32)
    assert node_input.shape == (N_TOTAL, DIM)
    w_full = _expand_w(np.asarray(affine_weight, dtype=np.float32)).reshape(1, DIM)
    bias = np.ascontiguousarray(
        np.asarray(affine_bias, dtype=np.float32).reshape(1, NS))

    in_maps = []
    for c in range(N_CORES):
        shard = np.ones((ROWS_PAD, DIM), dtype=np.float32)
        shard[:ROWS_PER_CORE] = node_input[c * ROWS_PER_CORE:(c + 1) * ROWS_PER_CORE]
        in_maps.append({"x": shard, "w": w_full, "b": bias})

    nc = _get_nc(ROWS_PAD, G)
    res = run_bass_kernel_spmd(nc, in_maps, core_ids=list(range(N_CORES)))
    out = np.concatenate(
        [np.asarray(res.results[c]["y"])[:ROWS_PER_CORE] for c in range(N_CORES)],
        axis=0)
    return out.astype(np.float32, copy=False)



# revision 4
# speedup vs baseline: 1.4300x; 1.4300x over previous
"""Trainium2 Bass kernel: EquivariantLayerNorm (irreps 128x0e + 64x1o + 32x2e).

Math (per row x of 480 features; scalar channels = first 128):
    m    = mean(x[:128]);  x'[:128] = x[:128] - m;  x'[128:] = x[128:]
    ss   = sum(x'^2)  = M2(x[:128]) + sum(x[128:]^2)
    r    = 1/sqrt(ss/224)
    y    = x' * r * w_full;  y[:128] += bias
The Invariant eps terms (eps=1e-6) contribute <1e-6 relative error and are
dropped (below fp32 rounding of the reference itself).

I/O in fp16 (host casts fp32->fp16 and back): halves HBM traffic (the
kernel is memory-bound) and doubles DVE throughput.  All statistics are
accumulated in fp32 on-chip; end-to-end rel err vs the fp32 reference is
~8e-4 against a 2e-2 budget.

Sharding: pure data-parallel over rows across 8 NeuronCores; each core gets
12500 rows padded to 12544 = 98 blocks of 128 rows (pads filled with 1.0).

Per-core layout: rows on partitions (128/block), features on the free dim.
Tiles of g blocks; DRAM<->SBUF via the "(p g) d" mapping so each partition
moves one contiguous (g*960)B run per DMA.

Engine split per tile (chosen from the v2 cost model + trace measurements):
  DVE:  bn_stats over the scalar block, batched [128,g,128]->[128,g,6]
        (gives mean AND centered M2 in one 1x pass); small fp32 combines;
        apply = per-block 4x tensor_scalar ops + per-tile 2x tensor_tensor
        (scalar_tensor_tensor has no fast modes - avoided on the hot path)
  ACT:  per-block Square+accum over the 352 vector channels only (dead
        main-out to PSUM: cheaper fixed cost), plus one Sqrt per tile
  Emission is software-pipelined: apply of tile t-1 is emitted between
  stats and combine of tile t, so the in-order engine queues never block
  on the partner engine's current tile.
"""

import numpy as np

DIM = 480
NS = 128          # scalar (0e) channels
NV = DIM - NS     # vector channels (352)
NF = 224          # irrep instances
BLK = 128         # rows per block (partition dim)
N_CORES = 8

N_TOTAL = 100000
ROWS_PER_CORE = N_TOTAL // N_CORES    # 12500
NBLOCKS = 98                          # ceil(12500/128)
ROWS_PAD = NBLOCKS * BLK              # 12544

# Variable tile schedule: small first tile (stats start after a short load),
# small last tile (short apply+store drain), big middle tiles.
G_SCHEDULE = (8, 16, 16, 16, 16, 16, 10)   # sums to 98


def _expand_w(affine_weight):
    return np.concatenate([
        affine_weight[0:128],
        np.repeat(affine_weight[128:192], 3),
        np.repeat(affine_weight[192:224], 5),
    ]).astype(np.float32)


def _split_excess_waits(nc, dummy_sem):
    """walrus' TRN2 codegen allows at most ONE sync-wait command per engine
    instruction (S3D3_*_STRUCT).  Tile's wait assignment can emit 2+ — move
    the excess onto standalone InstEventSemaphore no-ops (same engine, placed
    immediately before), which is the same mechanism Tile's own barriers use.
    Each carries a dead increment of ``dummy_sem`` (CoreSim requires updates).
    """
    from concourse import mybir

    n = 0
    for fn in nc.m.functions:
        for blk in fn.blocks:
            out = []
            changed = False
            for inst in blk.instructions:
                si = inst.sync_info
                if si is not None and si.on_wait and len(si.on_wait) > 1:
                    waits = list(si.on_wait)
                    for w in waits[:-1]:
                        n += 1
                        ev = mybir.InstEventSemaphore(
                            name=f"I-evsplit-{n}", ins=[], outs=[])
                        ev.engine = inst.engine
                        ev.sync_info = mybir.SyncInfo(
                            on_wait=[w],
                            on_update=[mybir.SyncUpdate(
                                sync_type="semaphore", id=dummy_sem.num,
                                ant_name=dummy_sem.name,
                                update_mode="sem-inc", update_value=1,
                                update_reg=None)])
                        out.append(ev)
                    inst.sync_info = mybir.SyncInfo(
                        on_wait=[waits[-1]], on_update=list(si.on_update or []))
                    changed = True
                out.append(inst)
            if changed:
                blk.instructions = out
    return n


def build_nc(rows_pad, g, data_bufs=5):
    import concourse.bacc as bacc
    import concourse.tile as tile
    from concourse import mybir
    # Loads issue from the SP HWDGE ring, stores from the ACT HWDGE ring —
    # separate FIFO rings, so store waits never block load issue. Pin one
    # completion-semaphore lane per ring (lane = ring order = FIFO-exact),
    # which also keeps consumers at one DMAHW wait per ring.
    from concourse import tile_sem_assignment as _tsa
    if not getattr(_tsa.TileClockTick, "_lane_by_engine", False):
        _orig_assign = _tsa.TileClockTick._assign_tick

        def _assign_tick_lane_by_engine(self, inst):
            if isinstance(inst, _tsa.DMAInst) and not isinstance(
                    inst, _tsa.bass_isa.UserSyncedRemoteDMADescs):
                if inst.engine == mybir.EngineType.SP:
                    self.next_hw_dma_idx = 0
                elif inst.engine == mybir.EngineType.Activation:
                    self.next_hw_dma_idx = 1
            return _orig_assign(self, inst)

        _tsa.TileClockTick._assign_tick = _assign_tick_lane_by_engine
        _tsa.TileClockTick._lane_by_engine = True

    f16 = mybir.dt.float16
    f32 = mybir.dt.float32
    Alu = mybir.AluOpType
    Act = mybir.ActivationFunctionType
    X = mybir.AxisListType.X

    nblocks = rows_pad // BLK
    assert rows_pad % BLK == 0
    if isinstance(g, int):
        assert nblocks % g == 0
        gs = [g] * (nblocks // g)
    else:
        gs = list(g)
        assert sum(gs) == nblocks
    ntiles = len(gs)

    nc = bacc.Bacc("TRN2", target_bir_lowering=False, debug=False)
    evsem = nc.alloc_semaphore("evsplit_dummy")
    x = nc.dram_tensor("x", [rows_pad, DIM], f16, kind="ExternalInput").ap()
    w = nc.dram_tensor("w", [1, DIM], f16, kind="ExternalInput").ap()
    b = nc.dram_tensor("b", [1, NS], f16, kind="ExternalInput").ap()
    y = nc.dram_tensor("y", [rows_pad, DIM], f16, kind="ExternalOutput").ap()

    with tile.TileContext(nc) as tc:
        with (
            tc.tile_pool(name="const", bufs=1) as cpool,
            tc.tile_pool(name="data", bufs=data_bufs) as dpool,
            tc.tile_pool(name="stats", bufs=ntiles) as spool,
            tc.tile_pool(name="psum", bufs=1, space="PSUM") as ppool,
        ):
            w_t = cpool.tile([BLK, DIM], f16, name="w_t")
            nc.sync.dma_start(out=w_t[:], in_=w.broadcast_to([BLK, DIM]))
            b_t = cpool.tile([BLK, NS], f16, name="b_t")
            nc.sync.dma_start(out=b_t[:], in_=b.broadcast_to([BLK, NS]))

            dfp = ppool.tile([BLK, NV], f32, name="dfp")   # dead store (ACT)

            def stats_a(st):
                # Engine first-touches of the freshly loaded tile.
                t, g, r0, xt = st["t"], st["g"], st["r0"], st["xt"]
                bn = spool.tile([BLK, g, 6], f32, tag="bn", name=f"bn{t}")
                for j in range(g):   # walrus: bn_stats out must be exactly 6
                    nc.vector.bn_stats(out=bn[:, j, :], in_=xt[:, j, 0:NS])
                ssv = spool.tile([BLK, g], f32, tag="ssv", name=f"ssv{t}")
                for j in range(g):
                    nc.scalar.activation(
                        out=dfp[:], in_=xt[:, j, NS:DIM], func=Act.Square,
                        scale=1.0, accum_out=ssv[:, j:j + 1])
                st["bn"], st["ssv"] = bn, ssv

            def combine(st):
                # Merge even/odd bn_stats halves:
                #   mean = (m_e + m_o)/2
                #   M2   = M2_e + M2_o + 32*(m_e - m_o)^2
                #   vv   = M2 + ssv ;  r = 1/sqrt(vv/224)
                t, g, bn, ssv = st["t"], st["g"], st["bn"], st["ssv"]
                m_e, m_o = bn[:, :, 1], bn[:, :, 4]
                v_e, v_o = bn[:, :, 2], bn[:, :, 5]
                nmean = spool.tile([BLK, g], f32, tag="nmean", name=f"nmean{t}")
                nc.vector.tensor_tensor(out=nmean[:], in0=m_e, in1=m_o,
                                        op=Alu.add)
                nc.vector.tensor_scalar(
                    out=nmean[:], in0=nmean[:], scalar1=-0.5, scalar2=None,
                    op0=Alu.mult)
                d = spool.tile([BLK, g], f32, tag="d", name=f"d{t}")
                nc.vector.tensor_tensor(out=d[:], in0=m_e, in1=m_o,
                                        op=Alu.subtract)
                vv = spool.tile([BLK, g], f32, tag="vv", name=f"vv{t}")
                nc.vector.tensor_tensor(out=vv[:], in0=v_e, in1=v_o,
                                        op=Alu.add)
                e = spool.tile([BLK, g], f32, tag="e", name=f"e{t}")
                nc.vector.scalar_tensor_tensor(
                    out=e[:], in0=d[:], scalar=32.0, in1=d[:],
                    op0=Alu.mult, op1=Alu.mult)
                nc.vector.tensor_tensor(out=vv[:], in0=vv[:], in1=e[:],
                                        op=Alu.add)
                nc.vector.tensor_tensor(out=vv[:], in0=vv[:], in1=ssv[:],
                                        op=Alu.add)
                sq = spool.tile([BLK, g], f32, tag="sq", name=f"sq{t}")
                nc.scalar.activation(out=sq[:], in_=vv[:], func=Act.Sqrt,
                                     scale=1.0 / float(NF))
                rr = spool.tile([BLK, g], f32, tag="rr", name=f"rr{t}")
                nc.vector.reciprocal(out=rr[:], in_=sq[:])
                st["nmean"], st["rr"] = nmean, rr

            def apply_store(st):
                g, r0, xt = st["g"], st["r0"], st["xt"]
                nmean, rr = st["nmean"], st["rr"]
                for j in range(g):
                    nc.vector.tensor_scalar(
                        out=xt[:, j, 0:NS], in0=xt[:, j, 0:NS],
                        scalar1=nmean[:, j:j + 1], scalar2=rr[:, j:j + 1],
                        op0=Alu.add, op1=Alu.mult)
                    nc.vector.tensor_scalar(
                        out=xt[:, j, NS:DIM], in0=xt[:, j, NS:DIM],
                        scalar1=rr[:, j:j + 1], scalar2=None, op0=Alu.mult)
                w_bc = w_t[:].unsqueeze(1).broadcast_to([BLK, g, DIM])
                nc.vector.tensor_tensor(out=xt[:], in0=xt[:], in1=w_bc,
                                        op=Alu.mult)
                b_bc = b_t[:].unsqueeze(1).broadcast_to([BLK, g, NS])
                nc.vector.tensor_tensor(
                    out=xt[:, :, 0:NS], in0=xt[:, :, 0:NS], in1=b_bc,
                    op=Alu.add)
                dst = y[r0:r0 + g * BLK, :].rearrange("(p g) d -> p g d", p=BLK)
                nc.scalar.dma_start(out=dst, in_=xt[:])

            prev = None
            r0 = 0
            for t, g in enumerate(gs):
                src = x[r0:r0 + g * BLK, :].rearrange("(p g) d -> p g d", p=BLK)
                xt = dpool.tile([BLK, g, DIM], f16, tag="xt", name=f"xt{t}")
                nc.sync.dma_start(out=xt[:], in_=src)
                st = {"t": t, "g": g, "r0": r0, "xt": xt}
                stats_a(st)
                if prev is not None:
                    apply_store(prev)
                combine(st)
                prev = st
                r0 += g * BLK
            apply_store(prev)

    nc.compile()
    _split_excess_waits(nc, evsem)
    return nc


_NC_CACHE = {}


def _get_nc(rows_pad, g):
    key = (rows_pad, tuple(g) if not isinstance(g, int) else g)
    if key not in _NC_CACHE:
        _NC_CACHE[key] = build_nc(rows_pad, g)
    return _NC_CACHE[key]


def _stage_inputs(node_input, affine_weight, affine_bias):
    node_input = np.ascontiguousarray(node_input, dtype=np.float32)
    assert node_input.shape == (N_TOTAL, DIM)
    w_full = _expand_w(np.asarray(affine_weight, dtype=np.float32))
    w16 = w_full.astype(np.float16).reshape(1, DIM)
    b16 = np.asarray(affine_bias, dtype=np.float32).astype(
        np.float16).reshape(1, NS)
    in_maps = []
    for c in range(N_CORES):
        shard = np.ones((ROWS_PAD, DIM), dtype=np.float16)
        shard[:ROWS_PER_CORE] = node_input[
            c * ROWS_PER_CORE:(c + 1) * ROWS_PER_CORE]
        in_maps.append({"x": shard, "w": w16, "b": b16})
    return in_maps


def kernel(node_input, affine_weight, affine_bias):
    from concourse.bass_utils import run_bass_kernel_spmd

    in_maps = _stage_inputs(node_input, affine_weight, affine_bias)
    nc = _get_nc(ROWS_PAD, G_SCHEDULE)
    res = run_bass_kernel_spmd(nc, in_maps, core_ids=list(range(N_CORES)))
    out = np.concatenate(
        [np.asarray(res.results[c]["y"])[:ROWS_PER_CORE]
         for c in range(N_CORES)],
        axis=0)
    return out.astype(np.float32)


# revision 8
# speedup vs baseline: 1.5158x; 1.0600x over previous
"""Trainium2 Bass kernel: EquivariantLayerNorm (irreps 128x0e + 64x1o + 32x2e).

Math (per row x of 480 features; scalar channels = first 128):
    m    = mean(x[:128]);  x'[:128] = x[:128] - m;  x'[128:] = x[128:]
    ss   = sum(x'^2)  = M2(x[:128]) + sum(x[128:]^2)
    r    = 1/sqrt(ss/224)
    y    = x' * r * w_full;  y[:128] += bias
The Invariant eps terms (eps=1e-6) contribute <1e-6 relative error and are
dropped (below fp32 rounding of the reference itself).

I/O in fp16 (host casts fp32->fp16 and back): halves HBM traffic (the
kernel is memory-bound) and doubles DVE throughput.  All statistics are
accumulated in fp32 on-chip; end-to-end rel err vs the fp32 reference is
~8e-4 against a 2e-2 budget.

Sharding: pure data-parallel over rows across 8 NeuronCores; each core gets
12500 rows padded to 12544 = 98 blocks of 128 rows (pads filled with 1.0).

Per-core layout: rows on partitions (128/block), features on the free dim.
Tiles of g blocks; DRAM<->SBUF via the "(p g) d" mapping so each partition
moves one contiguous (g*960)B run per DMA.

Engine split per tile (chosen from the v2 cost model + trace measurements):
  DVE:  bn_stats over the scalar block, batched [128,g,128]->[128,g,6]
        (gives mean AND centered M2 in one 1x pass); small fp32 combines;
        apply = per-block 4x tensor_scalar ops + per-tile 2x tensor_tensor
        (scalar_tensor_tensor has no fast modes - avoided on the hot path)
  ACT:  per-block Square+accum over the 352 vector channels only (dead
        main-out to PSUM: cheaper fixed cost), plus one Sqrt per tile
  Emission is software-pipelined: apply of tile t-1 is emitted between
  stats and combine of tile t, so the in-order engine queues never block
  on the partner engine's current tile.
"""

import numpy as np

DIM = 480
NS = 128          # scalar (0e) channels
NV = DIM - NS     # vector channels (352)
NF = 224          # irrep instances
BLK = 128         # rows per block (partition dim)
N_CORES = 8

N_TOTAL = 100000
ROWS_PER_CORE = N_TOTAL // N_CORES    # 12500
NBLOCKS = 98                          # ceil(12500/128)
ROWS_PAD = NBLOCKS * BLK              # 12544

# Variable tile schedule: small first tiles (stats start after a short load),
# small last tile (short apply+store drain), big middle tiles.
G_SCHEDULE = (4, 12, 14, 15, 15, 15, 15, 8)   # sums to 98

# Fraction of each tile's blocks whose scalar-part apply runs on ACT
# (out_s = rr*x_s + nmean*rr via the activation affine) instead of DVE
# (2-scalar tensor_scalar).  Balances the two engines.
ACT_APPLY_FRAC = 0.43


def _expand_w(affine_weight):
    return np.concatenate([
        affine_weight[0:128],
        np.repeat(affine_weight[128:192], 3),
        np.repeat(affine_weight[192:224], 5),
    ]).astype(np.float32)


def _split_excess_waits(nc, dummy_sem):
    """walrus' TRN2 codegen allows at most ONE sync-wait command per engine
    instruction (S3D3_*_STRUCT).  Tile's wait assignment can emit 2+ — move
    the excess onto standalone InstEventSemaphore no-ops (same engine, placed
    immediately before), which is the same mechanism Tile's own barriers use.
    Each carries a dead increment of ``dummy_sem`` (CoreSim requires updates).
    """
    from concourse import mybir

    n = 0
    for fn in nc.m.functions:
        for blk in fn.blocks:
            out = []
            changed = False
            for inst in blk.instructions:
                si = inst.sync_info
                if si is not None and si.on_wait and len(si.on_wait) > 1:
                    waits = list(si.on_wait)
                    for w in waits[:-1]:
                        n += 1
                        ev = mybir.InstEventSemaphore(
                            name=f"I-evsplit-{n}", ins=[], outs=[])
                        ev.engine = inst.engine
                        ev.sync_info = mybir.SyncInfo(
                            on_wait=[w],
                            on_update=[mybir.SyncUpdate(
                                sync_type="semaphore", id=dummy_sem.num,
                                ant_name=dummy_sem.name,
                                update_mode="sem-inc", update_value=1,
                                update_reg=None)])
                        out.append(ev)
                    inst.sync_info = mybir.SyncInfo(
                        on_wait=[waits[-1]], on_update=list(si.on_update or []))
                    changed = True
                out.append(inst)
            if changed:
                blk.instructions = out
    return n


def build_nc(rows_pad, g, data_bufs=5):
    import concourse.bacc as bacc
    import concourse.tile as tile
    from concourse import mybir
    # Loads issue from the SP HWDGE ring, stores from the ACT HWDGE ring —
    # separate FIFO rings, so store waits never block load issue. Pin one
    # completion-semaphore lane per ring (lane = ring order = FIFO-exact),
    # which also keeps consumers at one DMAHW wait per ring.
    from concourse import tile_sem_assignment as _tsa
    if not getattr(_tsa.TileClockTick, "_lane_by_engine", False):
        _orig_assign = _tsa.TileClockTick._assign_tick

        def _assign_tick_lane_by_engine(self, inst):
            if isinstance(inst, _tsa.DMAInst) and not isinstance(
                    inst, _tsa.bass_isa.UserSyncedRemoteDMADescs):
                if inst.engine == mybir.EngineType.SP:
                    self.next_hw_dma_idx = 0
                elif inst.engine == mybir.EngineType.Activation:
                    self.next_hw_dma_idx = 1
            return _orig_assign(self, inst)

        _tsa.TileClockTick._assign_tick = _assign_tick_lane_by_engine
        _tsa.TileClockTick._lane_by_engine = True

    f16 = mybir.dt.float16
    f32 = mybir.dt.float32
    Alu = mybir.AluOpType
    Act = mybir.ActivationFunctionType
    X = mybir.AxisListType.X

    nblocks = rows_pad // BLK
    assert rows_pad % BLK == 0
    if isinstance(g, int):
        assert nblocks % g == 0
        gs = [g] * (nblocks // g)
    else:
        gs = list(g)
        assert sum(gs) == nblocks
    ntiles = len(gs)

    nc = bacc.Bacc("TRN2", target_bir_lowering=False, debug=False)
    evsem = nc.alloc_semaphore("evsplit_dummy")
    x = nc.dram_tensor("x", [rows_pad, DIM], f16, kind="ExternalInput").ap()
    w = nc.dram_tensor("w", [1, DIM], f16, kind="ExternalInput").ap()
    b = nc.dram_tensor("b", [1, NS], f16, kind="ExternalInput").ap()
    y = nc.dram_tensor("y", [rows_pad, DIM], f16, kind="ExternalOutput").ap()

    with tile.TileContext(nc) as tc:
        with (
            tc.tile_pool(name="const", bufs=1) as cpool,
            tc.tile_pool(name="data", bufs=data_bufs) as dpool,
            tc.tile_pool(name="stats", bufs=ntiles) as spool,
            tc.tile_pool(name="psum", bufs=1, space="PSUM") as ppool,
        ):
            # w/b broadcast loads ride the ACT ring: their 128-descriptor
            # replication is slow and must not delay the first tile load on
            # the SP ring.
            w_t = cpool.tile([BLK, DIM], f16, name="w_t")
            nc.scalar.dma_start(out=w_t[:], in_=w.broadcast_to([BLK, DIM]))
            b_t = cpool.tile([BLK, NS], f16, name="b_t")
            nc.scalar.dma_start(out=b_t[:], in_=b.broadcast_to([BLK, NS]))

            dfp = ppool.tile([BLK, NV], f32, name="dfp")   # dead store (ACT)

            def stats_a(st):
                # Engine first-touches of the freshly loaded tile.
                t, g, r0, xt = st["t"], st["g"], st["r0"], st["xt"]
                bn = spool.tile([BLK, g, 6], f32, tag="bn", name=f"bn{t}")
                for j in range(g):   # walrus: bn_stats out must be exactly 6
                    nc.vector.bn_stats(out=bn[:, j, :], in_=xt[:, j, 0:NS])
                ssv = spool.tile([BLK, g], f32, tag="ssv", name=f"ssv{t}")
                for j in range(g):
                    nc.scalar.activation(
                        out=dfp[:], in_=xt[:, j, NS:DIM], func=Act.Square,
                        scale=1.0, accum_out=ssv[:, j:j + 1])
                st["bn"], st["ssv"] = bn, ssv

            def combine(st):
                # Merge even/odd bn_stats halves:
                #   mean = (m_e + m_o)/2
                #   M2   = M2_e + M2_o + 32*(m_e - m_o)^2
                #   vv   = M2 + ssv ;  r = 1/sqrt(vv/224)
                t, g, bn, ssv = st["t"], st["g"], st["bn"], st["ssv"]
                m_e, m_o = bn[:, :, 1], bn[:, :, 4]
                v_e, v_o = bn[:, :, 2], bn[:, :, 5]
                nmean = spool.tile([BLK, g], f32, tag="nmean", name=f"nmean{t}")
                nc.vector.tensor_tensor(out=nmean[:], in0=m_e, in1=m_o,
                                        op=Alu.add)
                nc.vector.tensor_scalar(
                    out=nmean[:], in0=nmean[:], scalar1=-0.5, scalar2=None,
                    op0=Alu.mult)
                d = spool.tile([BLK, g], f32, tag="d", name=f"d{t}")
                nc.vector.tensor_tensor(out=d[:], in0=m_e, in1=m_o,
                                        op=Alu.subtract)
                vv = spool.tile([BLK, g], f32, tag="vv", name=f"vv{t}")
                nc.vector.tensor_tensor(out=vv[:], in0=v_e, in1=v_o,
                                        op=Alu.add)
                e = spool.tile([BLK, g], f32, tag="e", name=f"e{t}")
                nc.vector.scalar_tensor_tensor(
                    out=e[:], in0=d[:], scalar=32.0, in1=d[:],
                    op0=Alu.mult, op1=Alu.mult)
                nc.vector.tensor_tensor(out=vv[:], in0=vv[:], in1=e[:],
                                        op=Alu.add)
                nc.vector.tensor_tensor(out=vv[:], in0=vv[:], in1=ssv[:],
                                        op=Alu.add)
                sq = spool.tile([BLK, g], f32, tag="sq", name=f"sq{t}")
                nc.scalar.activation(out=sq[:], in_=vv[:], func=Act.Sqrt,
                                     scale=1.0 / float(NF))
                rr = spool.tile([BLK, g], f32, tag="rr", name=f"rr{t}")
                nc.vector.reciprocal(out=rr[:], in_=sq[:])
                nmrr = spool.tile([BLK, g], f32, tag="nmrr", name=f"nmrr{t}")
                nc.vector.tensor_tensor(out=nmrr[:], in0=nmean[:], in1=rr[:],
                                        op=Alu.mult)
                st["nmean"], st["rr"], st["nmrr"] = nmean, rr, nmrr

            def apply_store(st):
                g, r0, xt = st["g"], st["r0"], st["xt"]
                nmean, rr, nmrr = st["nmean"], st["rr"], st["nmrr"]
                k = int(round(g * ACT_APPLY_FRAC))
                for j in range(g):
                    if j < k:
                        # scalar-part apply on ACT: rr*x_s + nmean*rr
                        nc.scalar.activation(
                            out=xt[:, j, 0:NS], in_=xt[:, j, 0:NS],
                            func=Act.Identity, bias=nmrr[:, j:j + 1],
                            scale=rr[:, j:j + 1])
                    else:
                        nc.vector.tensor_scalar(
                            out=xt[:, j, 0:NS], in0=xt[:, j, 0:NS],
                            scalar1=nmean[:, j:j + 1], scalar2=rr[:, j:j + 1],
                            op0=Alu.add, op1=Alu.mult)
                    nc.vector.tensor_scalar(
                        out=xt[:, j, NS:DIM], in0=xt[:, j, NS:DIM],
                        scalar1=rr[:, j:j + 1], scalar2=None, op0=Alu.mult)
                w_bc = w_t[:].unsqueeze(1).broadcast_to([BLK, g, DIM])
                nc.vector.tensor_tensor(out=xt[:], in0=xt[:], in1=w_bc,
                                        op=Alu.mult)
                b_bc = b_t[:].unsqueeze(1).broadcast_to([BLK, g, NS])
                nc.vector.tensor_tensor(
                    out=xt[:, :, 0:NS], in0=xt[:, :, 0:NS], in1=b_bc,
                    op=Alu.add)

            def store(st):
                g, r0, xt = st["g"], st["r0"], st["xt"]
                dst = y[r0:r0 + g * BLK, :].rearrange("(p g) d -> p g d", p=BLK)
                nc.scalar.dma_start(out=dst, in_=xt[:])

            # Emission order per iteration shapes each in-order engine queue:
            #   ACT: [apply-affine(t-1) x k, squares(t) x g, store-disp(t-1),
            #         sqrt(t)]
            #   DVE: [TS apply(t-1), TT_w/TT_b(t-1), bn(t) x g, combine(t)]
            # so no engine ever waits mid-queue on the partner's current tile.
            prev = None
            r0 = 0
            for t, g in enumerate(gs):
                src = x[r0:r0 + g * BLK, :].rearrange("(p g) d -> p g d", p=BLK)
                xt = dpool.tile([BLK, g, DIM], f16, tag="xt", name=f"xt{t}")
                nc.sync.dma_start(out=xt[:], in_=src)
                st = {"t": t, "g": g, "r0": r0, "xt": xt}
                if prev is not None:
                    apply_store(prev)
                stats_a(st)
                if prev is not None:
                    store(prev)
                combine(st)
                prev = st
                r0 += g * BLK
            apply_store(prev)
            store(prev)

    nc.compile()
    _split_excess_waits(nc, evsem)
    return nc


_NC_CACHE = {}


def _get_nc(rows_pad, g):
    key = (rows_pad, tuple(g) if not isinstance(g, int) else g)
    if key not in _NC_CACHE:
        _NC_CACHE[key] = build_nc(rows_pad, g)
    return _NC_CACHE[key]


def _stage_inputs(node_input, affine_weight, affine_bias):
    node_input = np.ascontiguousarray(node_input, dtype=np.float32)
    assert node_input.shape == (N_TOTAL, DIM)
    w_full = _expand_w(np.asarray(affine_weight, dtype=np.float32))
    w16 = w_full.astype(np.float16).reshape(1, DIM)
    b16 = np.asarray(affine_bias, dtype=np.float32).astype(
        np.float16).reshape(1, NS)
    in_maps = []
    for c in range(N_CORES):
        shard = np.ones((ROWS_PAD, DIM), dtype=np.float16)
        shard[:ROWS_PER_CORE] = node_input[
            c * ROWS_PER_CORE:(c + 1) * ROWS_PER_CORE]
        in_maps.append({"x": shard, "w": w16, "b": b16})
    return in_maps


def kernel(node_input, affine_weight, affine_bias):
    from concourse.bass_utils import run_bass_kernel_spmd

    in_maps = _stage_inputs(node_input, affine_weight, affine_bias)
    nc = _get_nc(ROWS_PAD, G_SCHEDULE)
    res = run_bass_kernel_spmd(nc, in_maps, core_ids=list(range(N_CORES)))
    out = np.concatenate(
        [np.asarray(res.results[c]["y"])[:ROWS_PER_CORE]
         for c in range(N_CORES)],
        axis=0)
    return out.astype(np.float32)
